# revision 5
# baseline (speedup 1.0000x reference)
"""DNPUConv2d Trainium2 kernel (8 NeuronCores, batch-parallel).

Restructure of the reference computation:
  - The per-device electrode permutation is folded into W1 by row
    permutation: z1 = u_d @ A_d + controls @ C_d, with A_d/C_d the
    data/control rows of the permuted W1.
  - Control contribution cb[o,i,d,:] = all_controls[o,i,d] @ C_d + b1 is
    precomputed on host (384 tiny vectors) and folded into the L1 matmul
    as an extra "ones row" of the rhs.
  - ELU via the exact identity elu(z) = max(min(exp(z),1), z+1) - 1.
    The "-1" is folded into the next layer's bias; biases ride as an
    extra ones-row through the matmuls, so on device each layer is
       z' = matmul (includes bias+1 row);  e = Exp(z'-1)  [ScalarE]
       g = (e min 1) max z'               [VectorE scalar_tensor_tensor]
  - The sum over (in_ch, device) is moved before the W3 dot (linearity):
    12 PSUM-accumulating matmuls per (b, o).
  - Two (i,d) combos are processed per [91, 512] tile; their L1 matmuls
    are merged into one K=8 matmul on zero-interleaved rhs columns.

Sharding: batch 16 -> 2 per core across 8 cores; weights replicated.
"""
import numpy as np

K = 3
PAD = 1
IN_CH = 8
OUT_CH = 16
DEV = 3
N_IN = 3
N_CTRL = 4
HID = 90
B = 16
HW = 16
L = HW * HW            # 256 output positions
N_CORES = 8
B_LOC = B // N_CORES   # 2 batches per core
M = HID + 1            # 91: hidden + ones row
NJ = IN_CH * DEV       # 24 (i,d) combos per o
NT = NJ // 2           # 12 tiles per (b,o), two combos each
X = 512                # tile free dim

_COMPILED = {}


def _build_program():
    import concourse.bacc as bacc
    import concourse.tile as tile
    from concourse import mybir

    f32 = mybir.dt.float32
    f32r = mybir.dt.float32r
    bf16 = mybir.dt.bfloat16
    Exp = mybir.ActivationFunctionType.Exp
    amin = mybir.AluOpType.min
    amax = mybir.AluOpType.max

    RCOLS = B_LOC * NT * X        # 12288 zero-interleaved rhs columns

    nc = bacc.Bacc()
    rhs1_d = nc.dram_tensor("rhs1", [8, RCOLS], f32r, kind="ExternalInput")
    lhs1_d = nc.dram_tensor("lhs1", [8, OUT_CH * NT, M], f32r,
                            kind="ExternalInput")
    w2g_d = nc.dram_tensor("w2g", [M, M], f32r, kind="ExternalInput")
    w3g_d = nc.dram_tensor("w3g", [M, 1], f32r, kind="ExternalInput")
    out_d = nc.dram_tensor("out", [B_LOC * OUT_CH, X], f32, kind="ExternalOutput")

    with tile.TileContext(nc) as tc:
        with (
            tc.tile_pool(name="singles", bufs=1) as singles,
            tc.tile_pool(name="work", bufs=4) as work,
            tc.tile_pool(name="outp", bufs=2) as outp,
            tc.tile_pool(name="psz1", bufs=3, space="PSUM") as psz1,
            tc.tile_pool(name="psz2", bufs=3, space="PSUM") as psz2,
            tc.tile_pool(name="psacc", bufs=2, space="PSUM") as psacc,
        ):
            rhs_sb = singles.tile([8, RCOLS], f32r)
            w2g_sb = singles.tile([M, M], f32r)
            w3g_sb = singles.tile([M, 1], f32r)
            neg1 = singles.tile([128, 1], f32)
            nc.vector.memset(neg1, -1.0)
            nc.sync.dma_start(out=rhs_sb, in_=rhs1_d[:, :])
            nc.sync.dma_start(out=w2g_sb, in_=w2g_d[:, :])
            nc.sync.dma_start(out=w3g_sb, in_=w3g_d[:, :])

            for o in range(OUT_CH):
                lhs_o = outp.tile([8, NT, M], f32r, tag="lhs_o", name="lhs_o")
                nc.sync.dma_start(out=lhs_o,
                                  in_=lhs1_d[:, o * NT:(o + 1) * NT, :])
                for b in range(B_LOC):
                    acc = psacc.tile([1, X], f32)
                    for t in range(NT):
                        col = (b * NT + t) * X
                        z1 = psz1.tile([M, X], f32)
                        nc.tensor.matmul(z1, lhs_o[:, t, :],
                                         rhs_sb[:, col:col + X],
                                         start=True, stop=True)
                        e1 = work.tile([M, X], f32, tag="e1")
                        nc.scalar.activation(e1, z1, Exp, bias=neg1[:M], scale=1.0)
                        g1 = work.tile([M, X], f32r, tag="g1")
                        nc.vector.scalar_tensor_tensor(
                            out=g1, in0=e1, scalar=1.0, in1=z1,
                            op0=amin, op1=amax)
                        z2 = psz2.tile([M, X], f32)
                        nc.tensor.matmul(z2, w2g_sb, g1, start=True, stop=True)
                        e2 = work.tile([M, X], f32, tag="e2")
                        nc.scalar.activation(e2, z2, Exp, bias=neg1[:M], scale=1.0)
                        g2 = work.tile([M, X], f32r, tag="g2")
                        nc.vector.scalar_tensor_tensor(
                            out=g2, in0=e2, scalar=1.0, in1=z2,
                            op0=amin, op1=amax)
                        nc.tensor.matmul(acc, w3g_sb, g2,
                                         start=(t == 0), stop=(t == NT - 1))
                    bo = b * OUT_CH + o
                    out_sb = outp.tile([1, X], f32, tag="osb", name="out_sb")
                    nc.vector.tensor_copy(out_sb, acc)
                    nc.sync.dma_start(out=out_d[bo:bo + 1, :], in_=out_sb)

    nc.compile()
    return nc


def _get_program():
    if "nc" not in _COMPILED:
        _COMPILED["nc"] = _build_program()
    return _COMPILED["nc"]


def _host_prep(x, all_controls, W1, b1, W2, b2, W3, b3,
               input_indices, control_indices):
    """Build per-core input maps; returns (in_maps, out_bias)."""
    x = np.asarray(x, np.float32)
    ac = np.asarray(all_controls, np.float32)
    W1 = np.asarray(W1, np.float32); b1 = np.asarray(b1, np.float32)
    W2 = np.asarray(W2, np.float32); b2 = np.asarray(b2, np.float32)
    W3 = np.asarray(W3, np.float32); b3 = np.asarray(b3, np.float32)
    ii = np.asarray(input_indices).astype(np.int64)
    ci = np.asarray(control_indices).astype(np.int64)

    # unfold (torch F.unfold ordering), pad=1, k=3, stride=1
    xp = np.pad(x, ((0, 0), (0, 0), (PAD, PAD), (PAD, PAD)))
    cols = [xp[:, :, i:i + HW, j:j + HW] for i in range(K) for j in range(K)]
    u = np.stack(cols, axis=2).reshape(B, IN_CH, K * K, L)
    u = u.transpose(0, 1, 3, 2).reshape(B, IN_CH, L, DEV, N_IN)

    # permuted W1 rows
    idx = np.concatenate([ii, ci], axis=-1)           # [DEV, 7]
    Wp = np.zeros((DEV, N_IN + N_CTRL, HID), np.float32)
    for d in range(DEV):
        for e in range(N_IN + N_CTRL):
            Wp[d, idx[d, e], :] = W1[e, :]
    A = Wp[:, :N_IN, :]                               # [DEV, 3, 90]
    C = Wp[:, N_IN:, :]                               # [DEV, 4, 90]
    cb = np.einsum('oidc,dch->oidh', ac, C) + b1      # [O, I, DEV, 90]
    b2f = b2 - W2.sum(axis=0)
    b3f = float((b3 - W3.sum(axis=0))[0])

    # lhs1: [8, O*NT, 91]; tile t covers combos (2t, 2t+1), j = i*DEV+d
    lhs1 = np.zeros((8, OUT_CH * NT, M), np.float32)
    for o in range(OUT_CH):
        for t in range(NT):
            ot = o * NT + t
            for h in range(2):
                j = 2 * t + h
                i, d = j // DEV, j % DEV
                r = 4 * h
                lhs1[r:r + N_IN, ot, :HID] = A[d]
                lhs1[r + N_IN, ot, :HID] = cb[o, i, d] + 1.0
                lhs1[r + N_IN, ot, HID] = 1.0
    w2g = np.zeros((M, M), np.float32)
    w2g[:HID, :HID] = W2
    w2g[HID, :HID] = b2f + 1.0
    w2g[HID, HID] = 1.0
    w3g = np.zeros((M, 1), np.float32)
    w3g[:HID, 0] = W3[:, 0]                           # b3f folded on host

    in_maps = []
    for c in range(N_CORES):
        ub = u[c * B_LOC:(c + 1) * B_LOC]             # [2, I, L, DEV, 3]
        rhs1 = np.zeros((8, B_LOC * NT * X), np.float32)
        for b in range(B_LOC):
            for t in range(NT):
                base = (b * NT + t) * X
                for h in range(2):
                    j = 2 * t + h
                    i, d = j // DEV, j % DEV
                    r = 4 * h
                    cs = base + h * L
                    rhs1[r:r + N_IN, cs:cs + L] = ub[b, i, :, d, :].T
                    rhs1[r + N_IN, cs:cs + L] = 1.0
        in_maps.append({"rhs1": rhs1, "lhs1": lhs1, "w2g": w2g, "w3g": w3g})
    return in_maps, NJ * b3f


def kernel(x, all_controls, W1, b1, W2, b2, W3, b3,
           input_indices, control_indices):
    from concourse.bass_utils import run_bass_kernel_spmd

    nc = _get_program()
    in_maps, out_bias = _host_prep(x, all_controls, W1, b1, W2, b2, W3, b3,
                                   input_indices, control_indices)
    res = run_bass_kernel_spmd(nc, in_maps, list(range(N_CORES)))
    out = np.empty((B, OUT_CH, HW, HW), np.float32)
    for c in range(N_CORES):
        o_c = res.results[c]["out"].reshape(B_LOC, OUT_CH, 2, L)
        out[c * B_LOC:(c + 1) * B_LOC] = (
            o_c[:, :, 0, :] + o_c[:, :, 1, :] + out_bias
        ).reshape(B_LOC, OUT_CH, HW, HW)
    return out
